# revision 36
# baseline (speedup 1.0000x reference)
"""Trainium2 Bass kernel for 4D valid convolution.

x (2,2,32,32,64,64) f32, weight (4,2,3,3,3,3) f32, bias (4,) f32
-> out (2,4,30,30,62,62) f32  (valid cross-correlation + bias)

Strategy: 8 cores = batch(2) x a-quadrant(4). Each core computes
out[b, :, a_sel, :, :, :] from slab x[b, :, a0:a0+10, :, :, :].

TensorE mapping per core (bf16 inputs, f32 PSUM accumulate):
  K (contraction, partitions) = (b-window=6, ci=2, a-window=10) = 120
  M (psum partitions)         = (co=4, a_out=8, b_out=4) = 128
  N (streamed free dim)       = contiguous (c,d) output pixels, <=496
Host prebuilds banded lhsT matrices (one per (k,l) tap, side by side in
one [120, 9*128] array -> a single DMA); the 9 (k,l) taps accumulate in
PSUM using (c,d)-shifted views of the same SBUF x tile, so each weight
load serves a full 496-column stream and the PE runs back-to-back at
~N cycles/matmul (209 ns/MM = the bf16 streaming roofline).

Startup/shutdown choreography (worth ~7 us vs the naive schedule):
 - 10 garbage warm-up matmuls (no input deps) keep the PE busy from the
   end of the framework preamble so the HAM clock gate reaches 8/8
   around the time the first real data lands (~12 us).
 - block 0 streams in FIVE c-stages sized so each chunk's data arrives
   just ahead of the PE: S1-S3+S5 ride Sync/ACT, S4 rides the GpSimd
   queue (idle after the weights); block 0 also runs 4-wide lead-in
   chunks matched to the stage sizes.
 - block 1 loads on GpSimd while Sync/ACT drain block 0's stages;
   the last block's weights transfer after them (not needed till bb7);
   blocks 2+ load with two collapsed "(b ci a) (c d)" descriptors each
   (slab staged host-side as [B2, CI, SA, C, D]).
 - the last block ends with a 2-wide chunk so the final eviction+store
   tail is short; stores alternate the ACT/Sync queues.

Measured: ~136.8-140 us HW exec (8 cores), max rel err ~2.2e-3 vs f32
reference (bf16 input rounding; PE idle < 1 us end to end).
"""

import sys

if "/opt/trn_rl_repo" not in sys.path:
    sys.path.insert(0, "/opt/trn_rl_repo")

import ml_dtypes
import numpy as np

BF16 = ml_dtypes.bfloat16

B, CI, CO = 2, 2, 4
A, B2, C, D = 32, 32, 64, 64
AO, BO, CL, DL = 30, 30, 62, 62
K = 3

# per-core a-slab starts; each core computes 8 output a-rows (q=3 overlaps q=2)
A0 = [0, 8, 16, 22]
SA = 10  # a-window (8 outputs + 2 halo)
SB = 6  # b-window per block (4 outputs + 2 halo)
NBB = 8  # b_out blocks: 7 full (4 wide) + 1 last (2 wide)
NCC = 8  # c chunks: 7 full (8 wide) + 1 last (6 wide)

_CACHE = {}


def _build_weights(weight: np.ndarray, bias: np.ndarray):
    """Banded lhsT matrices per (k,l) tap, plus per-partition bias vectors."""
    w = weight.astype(np.float32)

    def banded(sa, n_ao, sb, n_bo):
        # sel[d, o, t] = 1 if d == o + t
        sa_sel = np.zeros((sa, n_ao, K), np.float32)
        for t in range(K):
            for o in range(n_ao):
                sa_sel[o + t, o, t] = 1.0
        sb_sel = np.zeros((sb, n_bo, K), np.float32)
        for t in range(K):
            for o in range(n_bo):
                sb_sel[o + t, o, t] = 1.0
        # lhsT[(db,ci,da), t=(k,l), (co,ao,bo)] — taps side by side in columns
        # so the whole thing loads with a single 2D DMA into [P, 9*M]
        out = np.zeros((sb * CI * sa, 9, CO * n_ao * n_bo), np.float32)
        for k in range(K):
            for l in range(K):
                wkl = w[:, :, :, :, k, l]  # (co, ci, i, j)
                m = np.einsum("dai,ebj,ocij->ecdoab", sa_sel, sb_sel, wkl)
                out[:, k * 3 + l, :] = m.reshape(sb * CI * sa, CO * n_ao * n_bo)
        return np.ascontiguousarray(out.reshape(sb * CI * sa, 9 * CO * n_ao * n_bo))

    w_main = banded(SA, 8, SB, 4)  # (9, 120, 128)
    w_last = banded(SA, 8, 4, 2)  # (9, 80, 64)
    bias_main = np.repeat(bias.astype(np.float32), 32).reshape(128, 1)
    bias_last = np.repeat(bias.astype(np.float32), 16).reshape(64, 1)
    return w_main, w_last, bias_main, bias_last


def _build_program():
    import concourse.bass as bass  # noqa: F401
    import concourse.mybir as mybir
    import concourse.tile as tile
    from concourse import bacc

    f32 = mybir.dt.float32
    bf16 = mybir.dt.bfloat16

    nc = bacc.Bacc("TRN2", target_bir_lowering=False, debug=False, num_devices=8)
    xs = nc.dram_tensor("x_slab", [B2, CI, SA, C, D], bf16, kind="ExternalInput")
    wm = nc.dram_tensor("w_main", [120, 9 * 128], bf16, kind="ExternalInput")
    wl = nc.dram_tensor("w_last", [80, 9 * 64], bf16, kind="ExternalInput")
    bm = nc.dram_tensor("bias_main", [128, 1], f32, kind="ExternalInput")
    bl = nc.dram_tensor("bias_last", [64, 1], f32, kind="ExternalInput")
    # partition-major blocks: [bb, cc, m, n]; host unscrambles (cheap numpy)
    out = nc.dram_tensor(
        "out_blocks", [NBB, 128, CL * DL], f32, kind="ExternalOutput"
    )

    with tile.TileContext(nc) as tc:
        with (
            tc.tile_pool(name="w", bufs=1) as wpool,
            tc.tile_pool(name="rhs", bufs=8) as rpool,
            tc.tile_pool(name="psum", bufs=8, space="PSUM") as ppool,
            tc.tile_pool(name="ot", bufs=4) as opool,
        ):
            # weights ride the GpSimd DGE queue: off the critical rhs path
            w_main_t = wpool.tile([120, 9 * 128], bf16)
            nc.gpsimd.dma_start(w_main_t[:], wm[:])
            # PE warm-up: garbage matmuls (no input deps) lift the HAM
            # clock gate to 8/8 before the first real matmul arrives
            wu = wpool.tile([128, 512], bf16)
            nc.vector.memset(wu[:], 0)
            w_last_t = wpool.tile([80, 9 * 64], bf16)
            bias_main_t = wpool.tile([128, 1], f32)
            bias_last_t = wpool.tile([64, 1], f32)
            nc.gpsimd.dma_start(bias_main_t[:], bm[:])

            ps_wu = ppool.tile([128, 496], f32, tag="ps")
            for _ in range(10):
                nc.tensor.matmul(
                    ps_wu[:, :496], wu[:, :128], wu[:, :496], start=True, stop=True
                )

            # bb0 streams in 4 c-stages so the PE starts on stage 1
            # (~10.8us) and never waits again; chunk widths follow the
            # stages. bb7 ends with a tiny 2-wide chunk (short tail).
            STAGES = [(0, 6), (6, 10), (10, 18), (18, 30), (30, 46), (46, 64)]
            CH0 = [(0, 4), (4, 4), (8, 8), (16, 8), (24, 8), (32, 8), (40, 8), (48, 8), (56, 6)]
            CHM = [(0, 8), (8, 8), (16, 8), (24, 8), (32, 8), (40, 8), (48, 8), (56, 6)]
            CH7 = [(0, 8), (8, 8), (16, 8), (24, 8), (32, 8), (40, 8), (48, 8), (56, 4), (60, 2)]

            for bb in range(NBB):
                b0 = bb * 4
                wb = SB if bb < NBB - 1 else 4  # b-window width
                wbo = 4 if bb < NBB - 1 else 2  # b_out width
                P = CI * SA * wb  # 120 or 80
                M = CO * 8 * wbo  # 128 or 64

                rhs_t = rpool.tile([P, C * D], bf16, tag="rhs")
                h = wb // 2
                if bb == 0:
                    for si, (s0, s1) in enumerate(STAGES):
                        if si == 3:
                            # S4 rides the GpSimd queue (idle after the
                            # weights) so Sync/ACT drain S1-S3 + S5 in time
                            qs = ((0, h, nc.gpsimd), (h, wb, nc.gpsimd))
                        else:
                            qs = ((0, h, nc.sync), (h, wb, nc.scalar))
                        for lo, hi, q in qs:
                            q.dma_start(
                                rhs_t[lo * 20 : hi * 20, s0 * D : s1 * D],
                                xs[lo:hi, :, :, s0:s1].rearrange(
                                    "b ci a c d -> (b ci a) (c d)"
                                ),
                            )
                elif bb == 1:
                    # block 1 rides the GpSimd queue (idle after weights),
                    # so Sync/ACT keep draining block 0's stages; the
                    # last-block weights transfer after (needed at bb7)
                    for lo, hi in ((0, h), (h, wb)):
                        nc.gpsimd.dma_start(
                            rhs_t[lo * 20 : hi * 20, :],
                            xs[b0 + lo : b0 + hi].rearrange(
                                "b ci a c d -> (b ci a) (c d)"
                            ),
                        )
                    nc.gpsimd.dma_start(w_last_t[:], wl[:])
                    nc.gpsimd.dma_start(bias_last_t[:], bl[:])
                else:
                    for lo, hi, q in ((0, h, nc.sync), (h, wb, nc.scalar)):
                        q.dma_start(
                            rhs_t[lo * 20 : hi * 20, :],
                            xs[b0 + lo : b0 + hi].rearrange(
                                "b ci a c d -> (b ci a) (c d)"
                            ),
                        )
                rhs3 = rhs_t.rearrange("p (c d) -> p c d", c=C)
                w_t = w_main_t if bb < NBB - 1 else w_last_t
                bias_t = bias_main_t if bb < NBB - 1 else bias_last_t

                chunks = CH0 if bb == 0 else (CH7 if bb == NBB - 1 else CHM)
                for cc, (c0, wc) in enumerate(chunks):
                    N = wc * DL
                    ps = ppool.tile([M, N], f32, tag="ps")
                    for t in range(9):
                        k, l = divmod(t, 3)
                        rv = rhs3[:, c0 + k : c0 + k + wc, l : l + DL]
                        nc.tensor.matmul(
                            ps.rearrange("m (c d) -> m c d", c=wc),
                            w_t[:, t * M : (t + 1) * M],
                            rv,
                            start=(t == 0),
                            stop=(t == 8),
                        )
                    ot = opool.tile([M, N], f32, tag="ot")
                    nc.vector.tensor_scalar_add(ot[:], ps[:], bias_t[:M])
                    q = nc.scalar if cc % 2 == 0 else nc.sync
                    q.dma_start(out[bb, :M, c0 * DL : (c0 + wc) * DL], ot[:])

    nc.compile()
    return nc


def kernel(x: np.ndarray, weight: np.ndarray, bias: np.ndarray) -> np.ndarray:
    from concourse.bass_utils import run_bass_kernel_spmd

    if "nc" not in _CACHE:
        _CACHE["nc"] = _build_program()
    nc = _CACHE["nc"]

    w_main, w_last, bias_main, bias_last = _build_weights(weight, bias)
    x_bf = x.astype(BF16)
    w_main = w_main.astype(BF16)
    w_last = w_last.astype(BF16)

    in_maps = []
    for core in range(8):
        b, q = divmod(core, 4)
        a0 = A0[q]
        in_maps.append(
            {
                "x_slab": np.ascontiguousarray(
                    x_bf[b, :, a0 : a0 + SA].transpose(2, 0, 1, 3, 4)
                ),
                "w_main": w_main,
                "w_last": w_last,
                "bias_main": bias_main,
                "bias_last": bias_last,
            }
        )

    res = run_bass_kernel_spmd(nc, in_maps, core_ids=list(range(8)))
    _CACHE["last_result"] = res

    out = np.empty((B, CO, AO, BO, CL, DL), np.float32)
    for core in range(8):
        b, q = divmod(core, 4)
        slab = _unscramble(res.results[core]["out_blocks"])  # (4, 8, 30, 62, 62)
        if q < 3:
            out[b, :, 8 * q : 8 * q + 8] = slab
        else:
            out[b, :, 24:30] = slab[:, 2:8]
    return out


def _unscramble(blocks: np.ndarray) -> np.ndarray:
    """[NBB, 128, 62*62] partition-major blocks -> (4, 8, 30, 62, 62) slab."""
    slab = np.empty((CO, 8, BO, CL, DL), np.float32)
    for bb in range(NBB):
        wbo = 4 if bb < NBB - 1 else 2
        m = CO * 8 * wbo
        slab[:, :, bb * 4 : bb * 4 + wbo] = blocks[bb, :m].reshape(
            CO, 8, wbo, CL, DL
        )
    return slab


# revision 37
# speedup vs baseline: 1.0052x; 1.0052x over previous
"""Trainium2 Bass kernel for 4D valid convolution.

x (2,2,32,32,64,64) f32, weight (4,2,3,3,3,3) f32, bias (4,) f32
-> out (2,4,30,30,62,62) f32  (valid cross-correlation + bias)

Strategy: 8 cores = batch(2) x a-quadrant(4). Each core computes
out[b, :, a_sel, :, :, :] from slab x[b, :, a0:a0+10, :, :, :].

TensorE mapping per core (bf16 inputs, f32 PSUM accumulate):
  K (contraction, partitions) = (b-window=6, ci=2, a-window=10) = 120
  M (psum partitions)         = (co=4, a_out=8, b_out=4) = 128
  N (streamed free dim)       = contiguous (c,d) output pixels, <=496
Host prebuilds banded lhsT matrices (one per (k,l) tap, side by side in
one [120, 9*128] array -> a single DMA); the 9 (k,l) taps accumulate in
PSUM using (c,d)-shifted views of the same SBUF x tile, so each weight
load serves a full 496-column stream and the PE runs back-to-back at
~N cycles/matmul (209 ns/MM = the bf16 streaming roofline).

Startup/shutdown choreography (worth ~7 us vs the naive schedule):
 - 10 garbage warm-up matmuls (no input deps) keep the PE busy from the
   end of the framework preamble so the HAM clock gate reaches 8/8
   around the time the first real data lands (~12 us).
 - block 0 streams in FIVE c-stages sized so each chunk's data arrives
   just ahead of the PE: S1-S3+S5 ride Sync/ACT, S4 rides the GpSimd
   queue (idle after the weights); block 0 also runs 4-wide lead-in
   chunks matched to the stage sizes.
 - block 1 loads on GpSimd while Sync/ACT drain block 0's stages;
   the last block's weights transfer after them (not needed till bb7);
   blocks 2+ load with two collapsed "(b ci a) (c d)" descriptors each
   (slab staged host-side as [B2, CI, SA, C, D]).
 - the last block ends with a 2-wide chunk so the final eviction+store
   tail is short; stores alternate the ACT/Sync queues.

Measured: ~136.8-140 us HW exec (8 cores), max rel err ~2.2e-3 vs f32
reference (bf16 input rounding; PE idle < 1 us end to end).
"""

import sys

if "/opt/trn_rl_repo" not in sys.path:
    sys.path.insert(0, "/opt/trn_rl_repo")

import ml_dtypes
import numpy as np

BF16 = ml_dtypes.bfloat16

B, CI, CO = 2, 2, 4
A, B2, C, D = 32, 32, 64, 64
AO, BO, CL, DL = 30, 30, 62, 62
K = 3

# per-core a-slab starts; each core computes 8 output a-rows (q=3 overlaps q=2)
A0 = [0, 8, 16, 22]
SA = 10  # a-window (8 outputs + 2 halo)
SB = 6  # b-window per block (4 outputs + 2 halo)
NBB = 8  # b_out blocks: 7 full (4 wide) + 1 last (2 wide)
NCC = 8  # c chunks: 7 full (8 wide) + 1 last (6 wide)

_CACHE = {}


def _build_weights(weight: np.ndarray, bias: np.ndarray):
    """Banded lhsT matrices per (k,l) tap, plus per-partition bias vectors."""
    w = weight.astype(np.float32)

    def banded(sa, n_ao, sb, n_bo):
        # sel[d, o, t] = 1 if d == o + t
        sa_sel = np.zeros((sa, n_ao, K), np.float32)
        for t in range(K):
            for o in range(n_ao):
                sa_sel[o + t, o, t] = 1.0
        sb_sel = np.zeros((sb, n_bo, K), np.float32)
        for t in range(K):
            for o in range(n_bo):
                sb_sel[o + t, o, t] = 1.0
        # lhsT[(db,ci,da), t=(k,l), (co,ao,bo)] — taps side by side in columns
        # so the whole thing loads with a single 2D DMA into [P, 9*M]
        out = np.zeros((sb * CI * sa, 9, CO * n_ao * n_bo), np.float32)
        for k in range(K):
            for l in range(K):
                wkl = w[:, :, :, :, k, l]  # (co, ci, i, j)
                m = np.einsum("dai,ebj,ocij->ecdoab", sa_sel, sb_sel, wkl)
                out[:, k * 3 + l, :] = m.reshape(sb * CI * sa, CO * n_ao * n_bo)
        return np.ascontiguousarray(out.reshape(sb * CI * sa, 9 * CO * n_ao * n_bo))

    w_main = banded(SA, 8, SB, 4)  # (9, 120, 128)
    w_last = banded(SA, 8, 4, 2)  # (9, 80, 64)
    bias_main = np.repeat(bias.astype(np.float32), 32).reshape(128, 1)
    bias_last = np.repeat(bias.astype(np.float32), 16).reshape(64, 1)
    return w_main, w_last, bias_main, bias_last


def _build_program():
    import concourse.bass as bass  # noqa: F401
    import concourse.mybir as mybir
    import concourse.tile as tile
    from concourse import bacc

    f32 = mybir.dt.float32
    bf16 = mybir.dt.bfloat16

    nc = bacc.Bacc("TRN2", target_bir_lowering=False, debug=False, num_devices=8)
    xs = nc.dram_tensor("x_slab", [B2, CI, SA, C, D], bf16, kind="ExternalInput")
    wm = nc.dram_tensor("w_main", [120, 9 * 128], bf16, kind="ExternalInput")
    wl = nc.dram_tensor("w_last", [80, 9 * 64], bf16, kind="ExternalInput")
    bm = nc.dram_tensor("bias_main", [128, 1], f32, kind="ExternalInput")
    bl = nc.dram_tensor("bias_last", [64, 1], f32, kind="ExternalInput")
    # partition-major blocks: [bb, cc, m, n]; host unscrambles (cheap numpy)
    out = nc.dram_tensor(
        "out_blocks", [NBB, 128, CL * DL], f32, kind="ExternalOutput"
    )

    with tile.TileContext(nc) as tc:
        with (
            tc.tile_pool(name="w", bufs=1) as wpool,
            tc.tile_pool(name="rhs", bufs=8) as rpool,
            tc.tile_pool(name="psum", bufs=8, space="PSUM") as ppool,
            tc.tile_pool(name="ot", bufs=4) as opool,
        ):
            # weights ride the GpSimd DGE queue: off the critical rhs path
            w_main_t = wpool.tile([120, 9 * 128], bf16)
            nc.gpsimd.dma_start(w_main_t[:], wm[:])
            # PE warm-up: garbage matmuls (no input deps) lift the HAM
            # clock gate to 8/8 before the first real matmul arrives
            wu = wpool.tile([128, 512], bf16)
            nc.vector.memset(wu[:], 0)
            w_last_t = wpool.tile([80, 9 * 64], bf16)
            bias_main_t = wpool.tile([128, 1], f32)
            bias_last_t = wpool.tile([64, 1], f32)
            nc.gpsimd.dma_start(bias_main_t[:], bm[:])

            ps_wu = ppool.tile([128, 496], f32, tag="ps")
            for _ in range(10):
                nc.tensor.matmul(
                    ps_wu[:, :496], wu[:, :128], wu[:, :496], start=True, stop=True
                )

            # bb0 streams in 4 c-stages so the PE starts on stage 1
            # (~10.8us) and never waits again; chunk widths follow the
            # stages. bb7 ends with a tiny 2-wide chunk (short tail).
            STAGES = [(0, 6), (6, 14), (14, 26), (26, 42), (42, 64)]
            CH0 = [(0, 4), (4, 4), (8, 8), (16, 8), (24, 8), (32, 8), (40, 8), (48, 8), (56, 6)]
            CHM = [(0, 8), (8, 8), (16, 8), (24, 8), (32, 8), (40, 8), (48, 8), (56, 6)]
            CH7 = [(0, 8), (8, 8), (16, 8), (24, 8), (32, 8), (40, 8), (48, 8), (56, 4), (60, 2)]

            for bb in range(NBB):
                b0 = bb * 4
                wb = SB if bb < NBB - 1 else 4  # b-window width
                wbo = 4 if bb < NBB - 1 else 2  # b_out width
                P = CI * SA * wb  # 120 or 80
                M = CO * 8 * wbo  # 128 or 64

                rhs_t = rpool.tile([P, C * D], bf16, tag="rhs")
                h = wb // 2
                if bb == 0:
                    for si, (s0, s1) in enumerate(STAGES):
                        if si == 3:
                            # S4 rides the GpSimd queue (idle after the
                            # weights) so Sync/ACT drain S1-S3 + S5 in time
                            qs = ((0, h, nc.gpsimd), (h, wb, nc.gpsimd))
                        else:
                            qs = ((0, h, nc.sync), (h, wb, nc.scalar))
                        for lo, hi, q in qs:
                            q.dma_start(
                                rhs_t[lo * 20 : hi * 20, s0 * D : s1 * D],
                                xs[lo:hi, :, :, s0:s1].rearrange(
                                    "b ci a c d -> (b ci a) (c d)"
                                ),
                            )
                elif bb == 1:
                    # block 1 rides the GpSimd queue (idle after weights),
                    # so Sync/ACT keep draining block 0's stages; the
                    # last-block weights transfer after (needed at bb7)
                    for lo, hi in ((0, h), (h, wb)):
                        nc.gpsimd.dma_start(
                            rhs_t[lo * 20 : hi * 20, :],
                            xs[b0 + lo : b0 + hi].rearrange(
                                "b ci a c d -> (b ci a) (c d)"
                            ),
                        )
                    nc.gpsimd.dma_start(w_last_t[:], wl[:])
                    nc.gpsimd.dma_start(bias_last_t[:], bl[:])
                else:
                    for lo, hi, q in ((0, h, nc.sync), (h, wb, nc.scalar)):
                        q.dma_start(
                            rhs_t[lo * 20 : hi * 20, :],
                            xs[b0 + lo : b0 + hi].rearrange(
                                "b ci a c d -> (b ci a) (c d)"
                            ),
                        )
                rhs3 = rhs_t.rearrange("p (c d) -> p c d", c=C)
                w_t = w_main_t if bb < NBB - 1 else w_last_t
                bias_t = bias_main_t if bb < NBB - 1 else bias_last_t

                chunks = CH0 if bb == 0 else (CH7 if bb == NBB - 1 else CHM)
                for cc, (c0, wc) in enumerate(chunks):
                    N = wc * DL
                    ps = ppool.tile([M, N], f32, tag="ps")
                    for t in range(9):
                        k, l = divmod(t, 3)
                        rv = rhs3[:, c0 + k : c0 + k + wc, l : l + DL]
                        nc.tensor.matmul(
                            ps.rearrange("m (c d) -> m c d", c=wc),
                            w_t[:, t * M : (t + 1) * M],
                            rv,
                            start=(t == 0),
                            stop=(t == 8),
                        )
                    ot = opool.tile([M, N], f32, tag="ot")
                    nc.vector.tensor_scalar_add(ot[:], ps[:], bias_t[:M])
                    q = nc.scalar if cc % 2 == 0 else nc.sync
                    q.dma_start(out[bb, :M, c0 * DL : (c0 + wc) * DL], ot[:])

    nc.compile()
    return nc


def kernel(x: np.ndarray, weight: np.ndarray, bias: np.ndarray) -> np.ndarray:
    from concourse.bass_utils import run_bass_kernel_spmd

    if "nc" not in _CACHE:
        _CACHE["nc"] = _build_program()
    nc = _CACHE["nc"]

    w_main, w_last, bias_main, bias_last = _build_weights(weight, bias)
    x_bf = x.astype(BF16)
    w_main = w_main.astype(BF16)
    w_last = w_last.astype(BF16)

    in_maps = []
    for core in range(8):
        b, q = divmod(core, 4)
        a0 = A0[q]
        in_maps.append(
            {
                "x_slab": np.ascontiguousarray(
                    x_bf[b, :, a0 : a0 + SA].transpose(2, 0, 1, 3, 4)
                ),
                "w_main": w_main,
                "w_last": w_last,
                "bias_main": bias_main,
                "bias_last": bias_last,
            }
        )

    res = run_bass_kernel_spmd(nc, in_maps, core_ids=list(range(8)))
    _CACHE["last_result"] = res

    out = np.empty((B, CO, AO, BO, CL, DL), np.float32)
    for core in range(8):
        b, q = divmod(core, 4)
        slab = _unscramble(res.results[core]["out_blocks"])  # (4, 8, 30, 62, 62)
        if q < 3:
            out[b, :, 8 * q : 8 * q + 8] = slab
        else:
            out[b, :, 24:30] = slab[:, 2:8]
    return out


def _unscramble(blocks: np.ndarray) -> np.ndarray:
    """[NBB, 128, 62*62] partition-major blocks -> (4, 8, 30, 62, 62) slab."""
    slab = np.empty((CO, 8, BO, CL, DL), np.float32)
    for bb in range(NBB):
        wbo = 4 if bb < NBB - 1 else 2
        m = CO * 8 * wbo
        slab[:, :, bb * 4 : bb * 4 + wbo] = blocks[bb, :m].reshape(
            CO, 8, wbo, CL, DL
        )
    return slab
